# revision 12
# baseline (speedup 1.0000x reference)
"""Trainium2 Bass kernel for multi-head attention with symmetric ALiBi bias.

Computation (per batch n):
    v = (V.heads @ Wv.T), k = (K.heads @ Wk.T), q = (Q.heads @ Wq.T)
    energy[h,q,k] = q.k - slope_h*|tq-tk|, masked, softmax(/sqrt(E)), @v, @Wo.T + bo

Strategy: data parallel over batch N=16 -> 2 batches per core on 8 cores.
Algebra used inside the kernel:
  - energy = Q (Wq.T Wk) K.T: fold A=Wq.T@Wk into the K projection
    (ktil = K @ A.T), so Q needs no projection at all.
  - Work in transposed [kpos, q] layout so the mask bias is a per-partition
    vector fused into ACT's exp(x*scale + bias), and softmax max-subtraction
    is skipped (logits are provably < ~2 after the /32 scaling).
  - ALiBi factor exp(-slope*|q-k|/32) is Toeplitz: one [128, 2048] table per
    head covers every (kpos-tile, q) pair via a column offset.
  - A ones-column appended to v makes the softmax denominator fall out of the
    same matmul as attn@v; normalization is deferred to the tiny [64, q]
    per-head output and the 1/rowsum row is broadcast via a K=1 matmul.
  - fc_out: out @ Wo.T + bo computed from the transposed per-head outputs
    (which is exactly the lhsT layout the PE wants), bias via DVE add.
"""

import numpy as np

import concourse.bass as bass
import concourse.mybir as mybir
import concourse.tile as tile
from concourse.bass_utils import run_bass_kernel_spmd
from concourse.vector_clock import ScopedClock

F32 = mybir.dt.float32
F32R = mybir.dt.float32r
N_CORES = 8
NB = 2            # batches per core
L = 1024          # sequence length
E = 1024          # embed size
H = 16            # heads
D = 64            # head dim
P = 128           # partitions
LT = L // P       # 8 l-tiles
QC = 2            # q chunks of 512
FD = 512          # matmul free dim
NEG = -1.0e4 / 32.0  # mask fill after the /sqrt(E) fold
INV32 = 1.0 / 32.0


def _patch_tile_drain():
    """This container's walrus build rejects >2 sync waits on the Tile tail
    Drain; pre-emit the global-clock waits as single-wait instructions."""
    if getattr(tile.TileContext, "_drain_split_patch", False):
        return

    def _drain_and_barrier(self, tick_clock, wait_clock):
        nc = self.nc
        nop_inst = nc.sync.nop()
        wait_clock.add_sem_waits(
            nop_inst.ins, ScopedClock({None: tick_clock.global_clock})
        )
        waits = list(nop_inst.ins.sync_info.on_wait or [])
        if nop_inst.ins.sync_info is not None:
            nop_inst.ins.sync_info.on_wait = []
        assert self.sems is not None
        sem_by_num = {s.num: s for s in self.sems.allocated().values()}
        for w in waits:
            sem = sem_by_num.get(w.id)
            assert sem is not None and w.wait_mode == "sem-ge-imm", w
            nc.sync.wait_ge(sem, w.wait_value)
        nc.sync.drain()
        nc.all_engine_barrier()
        popped = nc._tile_sem_poison_stack.pop()
        assert popped is self._sem_poison
        nc.clear_and_free_semaphores(list(self.sems.allocated().values()))
        nc.all_engine_barrier()

    tile.TileContext._drain_and_barrier = _drain_and_barrier
    tile.TileContext._drain_split_patch = True


def _spill_excess_waits(nc, max_waits=1):
    """This container's walrus build rejects instructions carrying more than
    one sync wait. Move extras onto standalone event-semaphore waits inserted
    just before, on the same engine (engines execute their stream in order, so
    a preceding standalone wait is equivalent)."""
    import copy

    template = None
    for bb in nc.m.functions[0].blocks:
        for inst in bb.instructions:
            if type(inst).__name__ == "InstEventSemaphore":
                template = inst
                break
        if template is not None:
            break
    assert template is not None, "no InstEventSemaphore template found"

    for bb in nc.m.functions[0].blocks:
        new_insts = []
        for inst in bb.instructions:
            si = inst.sync_info
            if si is not None and si.on_wait and len(si.on_wait) > max_waits:
                excess = si.on_wait[: len(si.on_wait) - max_waits]
                keep = si.on_wait[len(si.on_wait) - max_waits :]
                for w in excess:
                    clone = copy.deepcopy(template)
                    clone.name = nc.get_next_instruction_name()
                    clone.engine = inst.engine
                    clone.sync_info = type(si)(on_wait=[w], on_update=[])
                    nc.register_instruction(clone, overwrite=True)
                    new_insts.append(clone)
                si.on_wait = keep
            new_insts.append(inst)
        bb.instructions[:] = new_insts


def _build_nc():
    _patch_tile_drain()
    nc = bass.Bass()
    qx = nc.declare_dram_parameter("qx", [NB, L, E], F32R, isOutput=False)
    kx = nc.declare_dram_parameter("kx", [NB, L, E], F32R, isOutput=False)
    vx = nc.declare_dram_parameter("vx", [NB, L, E], F32R, isOutput=False)
    mb = nc.declare_dram_parameter("mb", [NB, P, LT], F32, isOutput=False)
    d2 = nc.declare_dram_parameter("d2", [H, P, P], F32R, isOutput=False)
    us = nc.declare_dram_parameter("us", [H, P, LT], F32, isOutput=False)
    us2 = nc.declare_dram_parameter("us2", [H, P, LT], F32R, isOutput=False)
    w2r = nc.declare_dram_parameter("w2r", [H, D + 1, L], F32R, isOutput=False)
    at = nc.declare_dram_parameter("at", [D, D], F32R, isOutput=False)
    wvt = nc.declare_dram_parameter("wvt", [D, D], F32R, isOutput=False)
    wot = nc.declare_dram_parameter("wot", [LT, P, E], F32R, isOutput=False)
    bo = nc.declare_dram_parameter("bo", [E], F32, isOutput=False)
    iden = nc.declare_dram_parameter("iden", [P, P], F32R, isOutput=False)
    sel = nc.declare_dram_parameter("sel", [H, LT, P], F32, isOutput=False)
    out = nc.declare_dram_parameter("out", [NB, L, E], F32, isOutput=True)

    with tile.TileContext(nc) as tc:
        with (
            tc.tile_pool(name="const", bufs=1) as cpool,
            tc.tile_pool(name="expt", bufs=2) as epool,
            tc.tile_pool(name="inp", bufs=2) as ipool,
            tc.tile_pool(name="tr", bufs=2) as tpool,
            tc.tile_pool(name="attn", bufs=4) as apool,
            tc.tile_pool(name="abuf", bufs=10) as abpool,
            tc.tile_pool(name="lw", bufs=1) as lpool,
            tc.tile_pool(name="pe", bufs=2, space="PSUM") as pp_e,
            tc.tile_pool(name="po", bufs=2, space="PSUM") as pp_o,
            tc.tile_pool(name="pm", bufs=2, space="PSUM") as pp_m,
        ):
            iden_sb = cpool.tile([P, P], F32R)
            nc.sync.dma_start(out=iden_sb[:], in_=iden[:])
            sel_sb = cpool.tile([H, LT, P], F32)
            nc.sync.dma_start(out=sel_sb[:], in_=sel[:])
            at_sb = cpool.tile([D, D], F32R)
            nc.sync.dma_start(out=at_sb[:], in_=at[:])
            wvt_sb = cpool.tile([D, D], F32R)
            nc.sync.dma_start(out=wvt_sb[:], in_=wvt[:])
            wot_sb = cpool.tile([P, LT, E], F32R)
            nc.sync.dma_start(out=wot_sb[:], in_=wot[:].rearrange("j p e -> p j e"))
            bo_sb = cpool.tile([P, E], F32)
            nc.sync.dma_start(out=bo_sb[:], in_=bo[:][None, :].to_broadcast((P, E)))
            mb_sb = [
                cpool.tile([P, LT], F32, tag=f"mb{n}", name=f"mb_sb{n}")
                for n in range(NB)
            ]
            for n in range(NB):
                nc.sync.dma_start(out=mb_sb[n][:], in_=mb[n])

            for n in range(NB):
                # per-head transposed normalized outputs, laid out as Wo lhsT:
                # lw[p, j, q] = outTnorm[e = j*128 + p, q]
                lw = lpool.tile([P, LT, L], F32R, tag="lw")
                rs16 = lpool.tile([H, L], F32R, tag="rs16")
                for h in range(H):
                    d2_h = epool.tile([P, P], F32R, tag="d2")
                    nc.sync.dma_start(out=d2_h[:], in_=d2[h])
                    us_h = epool.tile([P, LT], F32, tag="us")
                    nc.sync.dma_start(out=us_h[:], in_=us[h])
                    us2_h = epool.tile([P, LT], F32R, tag="us2")
                    nc.sync.dma_start(out=us2_h[:], in_=us2[h])
                    w2r_h = epool.tile([D + 1, L], F32R, tag="w2r")
                    nc.sync.dma_start(out=w2r_h[:], in_=w2r[h])

                    qh = ipool.tile([P, LT, D], F32R, tag="qh")
                    kh = ipool.tile([P, LT, D], F32R, tag="kh")
                    vh = ipool.tile([P, LT, D], F32R, tag="vh")
                    hsl = slice(h * D, (h + 1) * D)
                    for src, dst in ((qx, qh), (kx, kh), (vx, vh)):
                        nc.sync.dma_start(
                            out=dst[:],
                            in_=src[n].rearrange("(lt p) e -> p lt e", p=P)[:, :, hsl],
                        )

                    # transpose to [d, l] via PE
                    qt = tpool.tile([D, L], F32R, tag="qt")
                    kt = tpool.tile([D, L], F32R, tag="kt")
                    vt = tpool.tile([D, L], F32R, tag="vt")
                    for tsrc, tdst, teng in (
                        (qh, qt, "s"),
                        (kh, kt, "s"),
                        (vh, vt, "v"),
                    ):
                        for g in range(2):
                            pt = pp_m.tile([D, FD], F32R, tag="m")
                            for i in range(4):
                                lt = g * 4 + i
                                nc.tensor.transpose(
                                    pt[:, i * P : (i + 1) * P],
                                    tsrc[:, lt, :],
                                    iden_sb[:],
                                )
                            dsl = tdst[:, g * FD : (g + 1) * FD]
                            if teng == "s":
                                nc.scalar.copy(dsl, pt[:])
                            else:
                                nc.vector.tensor_copy(dsl, pt[:])

                    # ktil.T = A.T-projected K.T ; energy contraction over raw dim
                    ktld = tpool.tile([D, L], F32R, tag="ktld")
                    for c in range(QC):
                        pk = pp_m.tile([D, FD], F32, tag="m")
                        nc.tensor.matmul(
                            pk[:],
                            at_sb[:],
                            kt[:, c * FD : (c + 1) * FD],
                        )
                        nc.scalar.copy(ktld[:, c * FD : (c + 1) * FD], pk[:])

                    # vhu[kpos, 0:64] = u[kpos] * (V @ Wv.T), col 64 = u
                    # (rowsum trick); vhup the same with u' = 1/u
                    pv = pp_m.tile([P, FD], F32, tag="m")
                    for lt in range(LT):
                        nc.tensor.matmul(
                            pv[:, lt * D : (lt + 1) * D],
                            vt[:, lt * P : (lt + 1) * P],
                            wvt_sb[:],
                        )
                    vhu = tpool.tile([P, LT, D + 1], F32R, tag="vhu")
                    vhup = tpool.tile([P, LT, D + 1], F32R, tag="vhup")
                    nc.vector.tensor_tensor(
                        vhu[:, :, 0:D],
                        pv[:].rearrange("p (lt d) -> p lt d", d=D),
                        us_h[:, :, None].to_broadcast((P, LT, D)),
                        mybir.AluOpType.mult,
                    )
                    nc.vector.tensor_copy(vhu[:, :, D : D + 1], us_h[:, :, None])
                    nc.gpsimd.tensor_tensor(
                        vhup[:],
                        vhu[:],
                        us2_h[:, :, None].to_broadcast((P, LT, D + 1)),
                        mybir.AluOpType.mult,
                    )

                    stg = tpool.tile([D + 1, L], F32R, tag="stg")
                    for c in range(QC):
                        qsl = slice(c * FD, (c + 1) * FD)
                        ol = pp_o.tile([D + 1, FD], F32, tag="ol")
                        ou = pp_o.tile([D + 1, FD], F32, tag="ou")
                        a_tiles = []
                        for lt in range(LT):
                            ep = pp_e.tile([P, FD], F32, tag="ep")
                            nc.tensor.matmul(
                                ep[:],
                                ktld[:, lt * P : (lt + 1) * P],
                                qt[:, qsl],
                            )
                            a = abpool.tile([P, FD], F32R, tag="a")
                            nc.scalar.activation(
                                a[:],
                                ep[:],
                                mybir.ActivationFunctionType.Exp,
                                bias=mb_sb[n][:, lt : lt + 1],
                                scale=INV32,
                            )
                            # diagonal block: fold the exact ALiBi factor (and
                            # the 1/(u*w) correction for kpos>q) via d2
                            if 4 * c <= lt < 4 * (c + 1):
                                dc = lt * P - c * FD
                                deng = nc.vector if lt % 2 == 0 else nc.gpsimd
                                deng.tensor_tensor(
                                    a[:, dc : dc + P],
                                    a[:, dc : dc + P],
                                    d2_h[:],
                                    mybir.AluOpType.mult,
                                )
                            a_tiles.append(a)
                            # lower-triangle accumulation (q >= kpos), cols
                            # from this tile's diagonal onward
                            lc0 = max(0, lt * P - c * FD)
                            if lc0 < FD:
                                nc.tensor.matmul(
                                    ol[:, lc0:FD],
                                    vhu[:, lt, :],
                                    a[:, lc0:FD],
                                    start=(lt == 0),
                                    stop=(lt == LT - 1),
                                    skip_group_check=True,
                                )
                        # upper-triangle accumulation (kpos > q), descending so
                        # the first (widest) matmul initializes the psum
                        uw_max = min(FD, max(0, (LT - 1) * P - c * FD))
                        if uw_max < FD:
                            nc.vector.memset(ou[:, uw_max:FD], 0.0)
                        first = True
                        for lt in range(LT - 1, 0, -1):
                            uw = min(FD, lt * P - c * FD)
                            if uw <= 0:
                                continue
                            nc.tensor.matmul(
                                ou[:, 0:uw],
                                vhup[:, lt, :],
                                a_tiles[lt][:, 0:uw],
                                start=first,
                                stop=(lt == 1 or (lt - 1) * P - c * FD <= 0),
                                skip_group_check=True,
                            )
                            first = False
                        # combine: stg = ol + w2 * ou  (both scaled by 1/w[q],
                        # which cancels in the final normalization)
                        t2 = apool.tile([D + 1, FD], F32R, tag="t2")
                        nc.vector.tensor_tensor(
                            t2[:], ou[:], w2r_h[:, qsl], mybir.AluOpType.mult
                        )
                        nc.vector.tensor_tensor(
                            stg[:, qsl], ol[:], t2[:], mybir.AluOpType.add
                        )
                    # place unnormalized outT into Wo-lhsT layout (partition
                    # shift done by SBUF->SBUF DMA); stash the rowsum row
                    j, off_p = h // 2, (h % 2) * D
                    nc.sync.dma_start(
                        out=lw[off_p : off_p + D, j, :], in_=stg[0:D, :]
                    )
                    nc.sync.dma_start(
                        out=rs16[h : h + 1, :], in_=stg[D : D + 1, :]
                    )

                # batched normalization: lw[e, q] *= 1/rowsum[head(e), q],
                # broadcasting each head's reciprocal row over its 64
                # partitions with a K=16 selection matmul
                rcp16 = apool.tile([H, L], F32, tag="rcp")
                nc.vector.reciprocal(rcp16[:], rs16[:].bitcast(F32))
                for j in range(LT):
                    for c2 in range(QC):
                        rbp = pp_m.tile([P, FD], F32, tag="m")
                        nc.tensor.matmul(
                            rbp[:],
                            sel_sb[:, j, :],
                            rcp16[:, c2 * FD : (c2 + 1) * FD],
                        )
                        lsl = lw[:, j, c2 * FD : (c2 + 1) * FD]
                        nc.vector.tensor_tensor(
                            lsl, lsl, rbp[:], mybir.AluOpType.mult
                        )

                # fc_out: out[q, e'] = sum_e outTnorm[e, q] * WoT[e, e'] + bo
                for qt_i in range(LT):
                    for c2 in range(QC):
                        fp = pp_e.tile([P, FD], F32, tag="ep")
                        for j in range(LT):
                            nc.tensor.matmul(
                                fp[:],
                                lw[:, j, qt_i * P : (qt_i + 1) * P],
                                wot_sb[:, j, c2 * FD : (c2 + 1) * FD],
                                start=(j == 0),
                                stop=(j == LT - 1),
                            )
                        fsb = apool.tile([P, FD], F32, tag="a")
                        nc.vector.tensor_tensor(
                            fsb[:],
                            fp[:],
                            bo_sb[:, c2 * FD : (c2 + 1) * FD],
                            mybir.AluOpType.add,
                        )
                        nc.sync.dma_start(
                            out=out[n, qt_i * P : (qt_i + 1) * P,
                                    c2 * FD : (c2 + 1) * FD],
                            in_=fsb[:],
                        )
    _spill_excess_waits(nc)
    return nc


_NC_CACHE = None


def _get_nc():
    global _NC_CACHE
    if _NC_CACHE is None:
        _NC_CACHE = _build_nc()
    return _NC_CACHE


def _host_prep(values, keys, queries, mask):
    """Build the 8 per-core input maps (numpy only)."""
    values = np.ascontiguousarray(values, dtype=np.float32)
    keys = np.ascontiguousarray(keys, dtype=np.float32)
    queries = np.ascontiguousarray(queries, dtype=np.float32)
    maskbias = np.where(mask.reshape(16, L) == 0, np.float32(NEG), np.float32(0.0))
    # mb[n, p, t] for kpos = t*128 + p
    mb_all = maskbias.reshape(16, LT, P).transpose(0, 2, 1).copy()
    in_maps = []
    for c in range(N_CORES):
        sl = slice(c * NB, (c + 1) * NB)
        in_maps.append(
            {
                "qx": queries[sl],
                "kx": keys[sl],
                "vx": values[sl],
                "mb": np.ascontiguousarray(mb_all[sl]),
            }
        )
    return in_maps


def _sel_matrix():
    # sel[h, j, p] = 1 iff global row e = j*128 + p belongs to head h
    s = np.zeros((H, LT, P), dtype=np.float32)
    for h in range(H):
        j, off = h // 2, (h % 2) * D
        s[h, j, off : off + D] = 1.0
    return s


def _host_prep_shared(Wv, Wk, Wq, Wo, bo):
    slopes = 2.0 ** (-np.arange(1, H + 1, dtype=np.float64))
    s = slopes[:, None, None] * INV32  # [H,1,1]
    i = np.arange(P, dtype=np.float64)[None, :, None]
    qq = np.arange(P, dtype=np.float64)[None, None, :]
    d2 = np.exp(-2.0 * s * np.maximum(0.0, i - qq)).astype(np.float32)
    kpos = (np.arange(LT)[None, None, :] * P + np.arange(P)[None, :, None]).astype(
        np.float64
    )  # [1,P,LT]
    us = np.exp(s * kpos).astype(np.float32)
    us2 = np.exp(-2.0 * s * kpos).astype(np.float32)
    qpos = np.arange(L, dtype=np.float64)[None, None, :]
    w2 = np.exp(2.0 * s * qpos)  # [H,1,L]
    w2r = np.broadcast_to(w2, (H, D + 1, L)).astype(np.float32).copy()
    shared = {
        "d2": d2,
        "us": us,
        "us2": us2,
        "w2r": w2r,
        "at": np.ascontiguousarray((Wk.T @ Wq).astype(np.float32)),
        "wvt": np.ascontiguousarray(Wv.T.astype(np.float32)),
        "wot": np.ascontiguousarray(
            Wo.T.reshape(LT, P, E).astype(np.float32)
        ),
        "bo": np.ascontiguousarray(bo.astype(np.float32)),
        "iden": np.eye(P, dtype=np.float32),
        "sel": _sel_matrix(),
    }
    return shared


def make_in_maps(values, keys, queries, mask, Wv, Wk, Wq, Wo, bo):
    per_core = _host_prep(
        np.asarray(values), np.asarray(keys), np.asarray(queries), np.asarray(mask)
    )
    shared = _host_prep_shared(
        np.asarray(Wv), np.asarray(Wk), np.asarray(Wq), np.asarray(Wo),
        np.asarray(bo),
    )
    for m in per_core:
        m.update(shared)
    return per_core


def run_in_maps(in_maps, **kwargs):
    nc = _get_nc()
    return run_bass_kernel_spmd(nc, in_maps, core_ids=list(range(N_CORES)), **kwargs)


def kernel(values, keys, queries, mask, Wv, Wk, Wq, Wo, bo):
    in_maps = make_in_maps(values, keys, queries, mask, Wv, Wk, Wq, Wo, bo)
    res = run_in_maps(in_maps)
    return np.concatenate([r["out"] for r in res.results], axis=0)


# revision 17
# speedup vs baseline: 1.0239x; 1.0239x over previous
"""Trainium2 Bass kernel for multi-head attention with symmetric ALiBi bias.

Computation (per batch n):
    v = (V.heads @ Wv.T), k = (K.heads @ Wk.T), q = (Q.heads @ Wq.T)
    energy[h,q,k] = q.k - slope_h*|tq-tk|, masked, softmax(/sqrt(E)), @v, @Wo.T + bo

Strategy: data parallel over batch N=16 -> 2 batches per core on 8 cores.
Algebra used inside the kernel:
  - energy = Q (Wq.T Wk) K.T: fold A=Wq.T@Wk into the K projection
    (ktil = K @ A.T), so Q needs no projection at all.
  - Work in transposed [kpos, q] layout so the mask bias is a per-partition
    vector fused into ACT's exp(x*scale + bias), and softmax max-subtraction
    is skipped (logits are provably < ~2 after the /32 scaling).
  - ALiBi factor exp(-slope*|q-k|/32) is Toeplitz: one [128, 2048] table per
    head covers every (kpos-tile, q) pair via a column offset.
  - A ones-column appended to v makes the softmax denominator fall out of the
    same matmul as attn@v; normalization is deferred to the tiny [64, q]
    per-head output and the 1/rowsum row is broadcast via a K=1 matmul.
  - fc_out: out @ Wo.T + bo computed from the transposed per-head outputs
    (which is exactly the lhsT layout the PE wants), bias via DVE add.
"""

import numpy as np

import concourse.bass as bass
import concourse.mybir as mybir
import concourse.tile as tile
from concourse.bass_utils import run_bass_kernel_spmd
from concourse.vector_clock import ScopedClock

F32 = mybir.dt.float32
F32R = mybir.dt.float32r
N_CORES = 8
NB = 2            # batches per core
L = 1024          # sequence length
E = 1024          # embed size
H = 16            # heads
D = 64            # head dim
P = 128           # partitions
LT = L // P       # 8 l-tiles
QC = 2            # q chunks of 512
FD = 512          # matmul free dim
NEG = -1.0e4 / 32.0  # mask fill after the /sqrt(E) fold
INV32 = 1.0 / 32.0


def _patch_tile_drain():
    """This container's walrus build rejects >2 sync waits on the Tile tail
    Drain; pre-emit the global-clock waits as single-wait instructions."""
    if getattr(tile.TileContext, "_drain_split_patch", False):
        return

    def _drain_and_barrier(self, tick_clock, wait_clock):
        nc = self.nc
        nop_inst = nc.sync.nop()
        wait_clock.add_sem_waits(
            nop_inst.ins, ScopedClock({None: tick_clock.global_clock})
        )
        waits = list(nop_inst.ins.sync_info.on_wait or [])
        if nop_inst.ins.sync_info is not None:
            nop_inst.ins.sync_info.on_wait = []
        assert self.sems is not None
        sem_by_num = {s.num: s for s in self.sems.allocated().values()}
        for w in waits:
            sem = sem_by_num.get(w.id)
            assert sem is not None and w.wait_mode == "sem-ge-imm", w
            nc.sync.wait_ge(sem, w.wait_value)
        nc.sync.drain()
        nc.all_engine_barrier()
        popped = nc._tile_sem_poison_stack.pop()
        assert popped is self._sem_poison
        nc.clear_and_free_semaphores(list(self.sems.allocated().values()))
        nc.all_engine_barrier()

    tile.TileContext._drain_and_barrier = _drain_and_barrier
    tile.TileContext._drain_split_patch = True


def _spill_excess_waits(nc, max_waits=1):
    """This container's walrus build rejects instructions carrying more than
    one sync wait. Move extras onto standalone event-semaphore waits inserted
    just before, on the same engine (engines execute their stream in order, so
    a preceding standalone wait is equivalent)."""
    import copy

    template = None
    for bb in nc.m.functions[0].blocks:
        for inst in bb.instructions:
            if type(inst).__name__ == "InstEventSemaphore":
                template = inst
                break
        if template is not None:
            break
    assert template is not None, "no InstEventSemaphore template found"

    for bb in nc.m.functions[0].blocks:
        new_insts = []
        for inst in bb.instructions:
            si = inst.sync_info
            if si is not None and si.on_wait and len(si.on_wait) > max_waits:
                excess = si.on_wait[: len(si.on_wait) - max_waits]
                keep = si.on_wait[len(si.on_wait) - max_waits :]
                for w in excess:
                    clone = copy.deepcopy(template)
                    clone.name = nc.get_next_instruction_name()
                    clone.engine = inst.engine
                    clone.sync_info = type(si)(on_wait=[w], on_update=[])
                    nc.register_instruction(clone, overwrite=True)
                    new_insts.append(clone)
                si.on_wait = keep
            new_insts.append(inst)
        bb.instructions[:] = new_insts


def _build_nc():
    _patch_tile_drain()
    nc = bass.Bass()
    qx = nc.declare_dram_parameter("qx", [NB, L, E], F32R, isOutput=False)
    kx = nc.declare_dram_parameter("kx", [NB, L, E], F32R, isOutput=False)
    vx = nc.declare_dram_parameter("vx", [NB, L, E], F32R, isOutput=False)
    mb = nc.declare_dram_parameter("mb", [NB, P, LT], F32, isOutput=False)
    d2 = nc.declare_dram_parameter("d2", [H, P, P], F32R, isOutput=False)
    us = nc.declare_dram_parameter("us", [H, P, LT], F32, isOutput=False)
    us2 = nc.declare_dram_parameter("us2", [H, P, LT], F32R, isOutput=False)
    w2r = nc.declare_dram_parameter("w2r", [H, D + 1, L], F32R, isOutput=False)
    at = nc.declare_dram_parameter("at", [P, D], F32R, isOutput=False)
    wvt = nc.declare_dram_parameter("wvt", [P, D], F32R, isOutput=False)
    wot = nc.declare_dram_parameter("wot", [LT, P, E], F32R, isOutput=False)
    bo = nc.declare_dram_parameter("bo", [E], F32, isOutput=False)
    iden = nc.declare_dram_parameter("iden", [P, P], F32R, isOutput=False)
    sel = nc.declare_dram_parameter("sel", [H, LT, P], F32R, isOutput=False)
    out = nc.declare_dram_parameter("out", [NB, L, E], F32, isOutput=True)

    with tile.TileContext(nc) as tc:
        with (
            tc.tile_pool(name="const", bufs=1) as cpool,
            tc.tile_pool(name="expt", bufs=2) as epool,
            tc.tile_pool(name="inp", bufs=2) as ipool,
            tc.tile_pool(name="tr", bufs=2) as tpool,
            tc.tile_pool(name="attn", bufs=3) as apool,
            tc.tile_pool(name="abuf", bufs=9) as abpool,
            tc.tile_pool(name="lw", bufs=1) as lpool,
            tc.tile_pool(name="pe", bufs=2, space="PSUM") as pp_e,
            tc.tile_pool(name="po", bufs=2, space="PSUM") as pp_o,
            tc.tile_pool(name="pm", bufs=2, space="PSUM") as pp_m,
        ):
            iden_sb = cpool.tile([P, P], F32R)
            nc.sync.dma_start(out=iden_sb[:], in_=iden[:])
            sel_sb = cpool.tile([H, LT, P], F32R)
            nc.sync.dma_start(out=sel_sb[:], in_=sel[:])
            at_sb = cpool.tile([P, D], F32R)
            nc.sync.dma_start(out=at_sb[:], in_=at[:])
            wvt_sb = cpool.tile([P, D], F32R)
            nc.sync.dma_start(out=wvt_sb[:], in_=wvt[:])
            wot_sb = cpool.tile([P, LT, E], F32R)
            nc.sync.dma_start(out=wot_sb[:], in_=wot[:].rearrange("j p e -> p j e"))
            bo_sb = cpool.tile([P, E], F32)
            nc.sync.dma_start(out=bo_sb[:], in_=bo[:][None, :].to_broadcast((P, E)))
            mb_sb = [
                cpool.tile([P, LT], F32, tag=f"mb{n}", name=f"mb_sb{n}")
                for n in range(NB)
            ]
            for n in range(NB):
                nc.sync.dma_start(out=mb_sb[n][:], in_=mb[n])

            for n in range(NB):
                # per-head transposed normalized outputs, laid out as Wo lhsT:
                # lw[p, j, q] = outTnorm[e = j*128 + p, q]
                lw = lpool.tile([P, LT, L], F32R, tag="lw")
                rs16 = lpool.tile([H, L], F32R, tag="rs16")
                for hp in range(H // 2):
                    # two heads per pass: transposes and K-projection work on
                    # 128-row [d, l] tiles covering both heads; per-head slices
                    # use partition offsets 0/64
                    qh = ipool.tile([P, LT, P], F32R, tag="qh")
                    kh = ipool.tile([P, LT, P], F32R, tag="kh")
                    vh = ipool.tile([P, LT, P], F32R, tag="vh")
                    hsl = slice(hp * 2 * D, (hp + 1) * 2 * D)
                    for src_, dst in ((qx, qh), (kx, kh), (vx, vh)):
                        nc.sync.dma_start(
                            out=dst[:],
                            in_=src_[n].rearrange("(lt p) e -> p lt e", p=P)[:, :, hsl],
                        )

                    # transpose to [d, l] via PE (both heads at once)
                    qt = tpool.tile([P, L], F32R, tag="qt")
                    kt = tpool.tile([P, L], F32R, tag="kt")
                    vt = tpool.tile([P, L], F32R, tag="vt")
                    for tsrc, tdst, teng in (
                        (qh, qt, "s"),
                        (kh, kt, "v"),
                        (vh, vt, "v"),
                    ):
                        for g in range(2):
                            pt = pp_m.tile([P, FD], F32R, tag="m")
                            for i in range(4):
                                lt = g * 4 + i
                                nc.tensor.transpose(
                                    pt[:, i * P : (i + 1) * P],
                                    tsrc[:, lt, :],
                                    iden_sb[:],
                                )
                            dsl = tdst[:, g * FD : (g + 1) * FD]
                            if teng == "s":
                                nc.scalar.copy(dsl, pt[:])
                            else:
                                nc.vector.tensor_copy(dsl, pt[:])

                    # ktil.T = A.T-projected K.T for both heads
                    ktld = tpool.tile([P, L], F32R, tag="ktld")
                    for c in range(QC):
                        pk = pp_m.tile([P, FD], F32, tag="m")
                        nc.tensor.matmul(
                            pk[0:D, :],
                            at_sb[0:D, :],
                            kt[0:D, c * FD : (c + 1) * FD],
                        )
                        # f32r + psum column offset is rejected by this walrus
                        # build; run the second head's projection in fp32
                        nc.tensor.matmul(
                            pk[D:P, :],
                            at_sb[D:P, :].bitcast(F32),
                            kt[D:P, c * FD : (c + 1) * FD].bitcast(F32),
                        )
                        nc.scalar.copy(ktld[:, c * FD : (c + 1) * FD], pk[:])

                    for h in (2 * hp, 2 * hp + 1):
                        off = (h % 2) * D
                        d2_h = epool.tile([P, P], F32R, tag="d2")
                        nc.sync.dma_start(out=d2_h[:], in_=d2[h])
                        us_h = epool.tile([P, LT], F32, tag="us")
                        nc.sync.dma_start(out=us_h[:], in_=us[h])
                        us2_h = epool.tile([P, LT], F32R, tag="us2")
                        nc.sync.dma_start(out=us2_h[:], in_=us2[h])
                        w2r_h = epool.tile([D + 1, L], F32R, tag="w2r")
                        nc.sync.dma_start(out=w2r_h[:], in_=w2r[h])

                        # vhu[kpos, 0:64] = u[kpos] * (V @ Wv.T), col 64 = u
                        # (rowsum trick); vhup the same with u' = 1/u
                        pv = pp_m.tile([P, FD], F32, tag="m")
                        for lt in range(LT):
                            nc.tensor.matmul(
                                pv[:, lt * D : (lt + 1) * D],
                                vt[off : off + D, lt * P : (lt + 1) * P],
                                wvt_sb[off : off + D, :],
                            )
                        vhu = tpool.tile([P, LT, D + 1], F32R, tag="vhu")
                        vhup = tpool.tile([P, LT, D + 1], F32R, tag="vhup")
                        nc.vector.tensor_tensor(
                            vhu[:, :, 0:D],
                            pv[:].rearrange("p (lt d) -> p lt d", d=D),
                            us_h[:, :, None].to_broadcast((P, LT, D)),
                            mybir.AluOpType.mult,
                        )
                        nc.vector.tensor_copy(vhu[:, :, D : D + 1], us_h[:, :, None])
                        nc.gpsimd.tensor_tensor(
                            vhup[:],
                            vhu[:],
                            us2_h[:, :, None].to_broadcast((P, LT, D + 1)),
                            mybir.AluOpType.mult,
                        )

                        stg = tpool.tile([D + 1, L], F32R, tag="stg")
                        for c in range(QC):
                            qsl = slice(c * FD, (c + 1) * FD)
                            ol = pp_o.tile([D + 1, FD], F32, tag="ol")
                            ou = pp_o.tile([D + 1, FD], F32, tag="ou")
                            a_tiles = []
                            for lt in range(LT):
                                ep = pp_e.tile([P, FD], F32, tag="ep")
                                nc.tensor.matmul(
                                    ep[:],
                                    ktld[off : off + D, lt * P : (lt + 1) * P],
                                    qt[off : off + D, qsl],
                                )
                                a = abpool.tile([P, FD], F32R, tag="a")
                                nc.scalar.activation(
                                    a[:],
                                    ep[:],
                                    mybir.ActivationFunctionType.Exp,
                                    bias=mb_sb[n][:, lt : lt + 1],
                                    scale=INV32,
                                )
                                # diagonal block: exact ALiBi factor (plus the
                                # 1/(u*w) correction for kpos>q) via d2
                                if 4 * c <= lt < 4 * (c + 1):
                                    dc = lt * P - c * FD
                                    deng = nc.vector if lt % 2 == 0 else nc.gpsimd
                                    deng.tensor_tensor(
                                        a[:, dc : dc + P],
                                        a[:, dc : dc + P],
                                        d2_h[:],
                                        mybir.AluOpType.mult,
                                    )
                                a_tiles.append(a)
                                # lower-triangle accumulation (q >= kpos)
                                lc0 = max(0, lt * P - c * FD)
                                if lc0 < FD:
                                    nc.tensor.matmul(
                                        ol[:, lc0:FD],
                                        vhu[:, lt, :],
                                        a[:, lc0:FD],
                                        start=(lt == 0),
                                        stop=(lt == LT - 1),
                                        skip_group_check=True,
                                    )
                            # upper-triangle accumulation (kpos > q), descending
                            # so the first (widest) matmul initializes the psum
                            uw_max = min(FD, max(0, (LT - 1) * P - c * FD))
                            if uw_max < FD:
                                nc.vector.memset(ou[:, uw_max:FD], 0.0)
                            first = True
                            for lt in range(LT - 1, 0, -1):
                                uw = min(FD, lt * P - c * FD)
                                if uw <= 0:
                                    continue
                                nc.tensor.matmul(
                                    ou[:, 0:uw],
                                    vhup[:, lt, :],
                                    a_tiles[lt][:, 0:uw],
                                    start=first,
                                    stop=(lt == 1 or (lt - 1) * P - c * FD <= 0),
                                    skip_group_check=True,
                                )
                                first = False
                            # combine: stg = ol + w2 * ou (both scaled by
                            # 1/w[q], which cancels in the normalization)
                            t2 = apool.tile([D + 1, FD], F32R, tag="t2")
                            nc.vector.tensor_tensor(
                                t2[:], ou[:], w2r_h[:, qsl], mybir.AluOpType.mult
                            )
                            nc.vector.tensor_tensor(
                                stg[:, qsl], ol[:], t2[:], mybir.AluOpType.add
                            )
                        # place unnormalized outT into Wo-lhsT layout (partition
                        # shift done by SBUF->SBUF DMA); stash the rowsum row
                        j, off_p = h // 2, (h % 2) * D
                        nc.sync.dma_start(
                            out=lw[off_p : off_p + D, j, :], in_=stg[0:D, :]
                        )
                        nc.sync.dma_start(
                            out=rs16[h : h + 1, :], in_=stg[D : D + 1, :]
                        )

                # batched normalization: lw[e, q] *= 1/rowsum[head(e), q],
                # broadcasting each head's reciprocal row over its 64
                # partitions with a K=16 selection matmul
                rcp16 = apool.tile([H, L], F32R, tag="rcp")
                with nc.allow_low_precision(reason="normalization factor in f32r"):
                    nc.vector.reciprocal(rcp16[:], rs16[:].bitcast(F32))
                for j in range(LT):
                    for c2 in range(QC):
                        rbp = pp_m.tile([P, FD], F32, tag="m")
                        nc.tensor.matmul(
                            rbp[:],
                            sel_sb[:, j, :],
                            rcp16[:, c2 * FD : (c2 + 1) * FD],
                        )
                        lsl = lw[:, j, c2 * FD : (c2 + 1) * FD]
                        nc.vector.tensor_tensor(
                            lsl, lsl, rbp[:], mybir.AluOpType.mult
                        )

                # fc_out: out[q, e'] = sum_e outTnorm[e, q] * WoT[e, e'] + bo
                for qt_i in range(LT):
                    for c2 in range(QC):
                        fp = pp_e.tile([P, FD], F32, tag="ep")
                        for j in range(LT):
                            nc.tensor.matmul(
                                fp[:],
                                lw[:, j, qt_i * P : (qt_i + 1) * P],
                                wot_sb[:, j, c2 * FD : (c2 + 1) * FD],
                                start=(j == 0),
                                stop=(j == LT - 1),
                            )
                        fsb = apool.tile([P, FD], F32, tag="a")
                        nc.vector.tensor_tensor(
                            fsb[:],
                            fp[:],
                            bo_sb[:, c2 * FD : (c2 + 1) * FD],
                            mybir.AluOpType.add,
                        )
                        nc.sync.dma_start(
                            out=out[n, qt_i * P : (qt_i + 1) * P,
                                    c2 * FD : (c2 + 1) * FD],
                            in_=fsb[:],
                        )
    _spill_excess_waits(nc)
    return nc


_NC_CACHE = None


def _get_nc():
    global _NC_CACHE
    if _NC_CACHE is None:
        _NC_CACHE = _build_nc()
    return _NC_CACHE


def _host_prep(values, keys, queries, mask):
    """Build the 8 per-core input maps (numpy only)."""
    values = np.ascontiguousarray(values, dtype=np.float32)
    keys = np.ascontiguousarray(keys, dtype=np.float32)
    queries = np.ascontiguousarray(queries, dtype=np.float32)
    maskbias = np.where(mask.reshape(16, L) == 0, np.float32(NEG), np.float32(0.0))
    # mb[n, p, t] for kpos = t*128 + p
    mb_all = maskbias.reshape(16, LT, P).transpose(0, 2, 1).copy()
    in_maps = []
    for c in range(N_CORES):
        sl = slice(c * NB, (c + 1) * NB)
        in_maps.append(
            {
                "qx": queries[sl],
                "kx": keys[sl],
                "vx": values[sl],
                "mb": np.ascontiguousarray(mb_all[sl]),
            }
        )
    return in_maps


def _sel_matrix():
    # sel[h, j, p] = 1 iff global row e = j*128 + p belongs to head h
    s = np.zeros((H, LT, P), dtype=np.float32)
    for h in range(H):
        j, off = h // 2, (h % 2) * D
        s[h, j, off : off + D] = 1.0
    return s


def _host_prep_shared(Wv, Wk, Wq, Wo, bo):
    slopes = 2.0 ** (-np.arange(1, H + 1, dtype=np.float64))
    s = slopes[:, None, None] * INV32  # [H,1,1]
    i = np.arange(P, dtype=np.float64)[None, :, None]
    qq = np.arange(P, dtype=np.float64)[None, None, :]
    d2 = np.exp(-2.0 * s * np.maximum(0.0, i - qq)).astype(np.float32)
    kpos = (np.arange(LT)[None, None, :] * P + np.arange(P)[None, :, None]).astype(
        np.float64
    )  # [1,P,LT]
    us = np.exp(s * kpos).astype(np.float32)
    us2 = np.exp(-2.0 * s * kpos).astype(np.float32)
    qpos = np.arange(L, dtype=np.float64)[None, None, :]
    w2 = np.exp(2.0 * s * qpos)  # [H,1,L]
    w2r = np.broadcast_to(w2, (H, D + 1, L)).astype(np.float32).copy()
    shared = {
        "d2": d2,
        "us": us,
        "us2": us2,
        "w2r": w2r,
        "at": np.ascontiguousarray(np.tile((Wk.T @ Wq).astype(np.float32), (2, 1))),
        "wvt": np.ascontiguousarray(np.tile(Wv.T.astype(np.float32), (2, 1))),
        "wot": np.ascontiguousarray(
            Wo.T.reshape(LT, P, E).astype(np.float32)
        ),
        "bo": np.ascontiguousarray(bo.astype(np.float32)),
        "iden": np.eye(P, dtype=np.float32),
        "sel": _sel_matrix(),
    }
    return shared


def make_in_maps(values, keys, queries, mask, Wv, Wk, Wq, Wo, bo):
    per_core = _host_prep(
        np.asarray(values), np.asarray(keys), np.asarray(queries), np.asarray(mask)
    )
    shared = _host_prep_shared(
        np.asarray(Wv), np.asarray(Wk), np.asarray(Wq), np.asarray(Wo),
        np.asarray(bo),
    )
    for m in per_core:
        m.update(shared)
    return per_core


def run_in_maps(in_maps, **kwargs):
    nc = _get_nc()
    return run_bass_kernel_spmd(nc, in_maps, core_ids=list(range(N_CORES)), **kwargs)


def kernel(values, keys, queries, mask, Wv, Wk, Wq, Wo, bo):
    in_maps = make_in_maps(values, keys, queries, mask, Wv, Wk, Wq, Wo, bo)
    res = run_in_maps(in_maps)
    return np.concatenate([r["out"] for r in res.results], axis=0)
